# revision 16
# baseline (speedup 1.0000x reference)
import sys, os
sys.path.insert(0, '/opt/trn_rl_repo')
import numpy as np
import ml_dtypes
import concourse.bass as bass
import concourse.mybir as mybir
from concourse import tile
from concourse.vector_clock import ScopedClock
from concourse.bass_utils import run_bass_kernel_spmd

F32 = mybir.dt.float32
BF16 = mybir.dt.bfloat16
AF = mybir.ActivationFunctionType
NPBF = ml_dtypes.bfloat16

NCORE = 8
B, T, C, H, W, NCL = 128, 32, 512, 8, 32, 6625
S = B // NCORE          # 16 samples per core
HW = H * W              # 256
HH = C // 2             # 256
TB = T * S              # 512 cols, index 16*t + b
NCP = 6656              # padded num_class (13 * 512)

MAXW = 1
import bass_rust as _br
_NOPCLS = mybir.InstNoOp if hasattr(mybir, "InstNoOp") else getattr(_br, "InstNoOp")

class ChunkedTC(tile.TileContext):
    def _add_instruction(self, inst):
        si = getattr(inst, "sync_info", None)
        if si is not None and si.on_wait and len(si.on_wait) > MAXW:
            if os.environ.get("LOGSPLIT"):
                print("SPLIT", type(inst).__name__, inst.engine, len(si.on_wait), [w.ant_name for w in si.on_wait])
            waits = list(si.on_wait)
            head, tail = waits[:-MAXW], waits[-MAXW:]
            NopCls = type(inst).__mro__  # placeholder
            for k in range(0, len(head), MAXW):
                ch = head[k:k + MAXW]
                nop = _NOPCLS(name=self.nc.get_next_instruction_name(), ins=[], outs=[],
                              hint="waitsplit", nofuse=True)
                nop.engine = inst.engine
                nop.sync_info = mybir.SyncInfo(on_wait=ch, on_update=[])
                super()._add_instruction(nop)
            si.on_wait = tail
        super()._add_instruction(inst)

    def _drain_and_barrier(self, tick_clock, wait_clock):
        nc = self.nc
        nops = [nc.sync.nop(nofuse=True) for _ in range(24)]
        drain_inst = nc.sync.drain()
        wait_clock.add_sem_waits(drain_inst.ins, ScopedClock({None: tick_clock.global_clock}))
        si = drain_inst.ins.sync_info
        waits = list(si.on_wait) if si and si.on_wait else []
        if len(waits) > MAXW:
            chunks = [waits[i:i + MAXW] for i in range(0, len(waits), MAXW)]
            keep = chunks[-1]
            extra = chunks[:-1]
            assert len(extra) <= len(nops)
            for nop, ch in zip(nops, extra):
                nsi = nop.ins.sync_info
                if nsi is None:
                    nop.ins.sync_info = mybir.SyncInfo(on_wait=ch, on_update=[])
                else:
                    nsi.on_wait = ch
            si.on_wait = keep
        nc.all_engine_barrier()
        assert self.sems is not None
        popped = nc._tile_sem_poison_stack.pop()
        assert popped is self._sem_poison
        nc.clear_and_free_semaphores(list(self.sems.allocated().values()))
        nc.all_engine_barrier()


def build():
    nc = bass.Bass()
    dp = lambda n, sh, dt: nc.declare_dram_parameter(n, sh, dt, isOutput=False)
    featT = dp("featT", [S, 128, 2, C], BF16)         # per-sample [hw, c], hw split 2x128
    attm = dp("attm", [4, 128, HW], F32)              # rows (s%4, t)
    peT = dp("peT", [128, 4, TB], BF16)               # prev-emb.T chunks (xg rows 512-1023)
    wihT = dp("wihT", [128, 2, 4, 1024], BF16)        # per dir, c-chunks
    whhT = dp("whhT", [128, 2, 2, 1024], BF16)
    wgihT = dp("wgihT", [128, 8, 1536], BF16)
    wghhT = dp("wghhT", [128, 4, 1536], BF16)
    genwT = dp("genwT", [13, 128, 4, 512], BF16)      # [nchunk][cchunk][128,512]
    blstm = dp("blstm", [1, 2, 1024], BF16)
    bgih = dp("bgih", [1, 1536], BF16)
    bghh = dp("bghh", [1, 1536], BF16)
    genb = dp("genb", [1, NCP], BF16)
    ident = dp("ident", [128, 128], F32)
    logits = nc.declare_dram_parameter("logits", [TB, NCL], F32, isOutput=True)
    attout = nc.declare_dram_parameter("attout", [TB, HW], F32, isOutput=True)
    dbg = nc.declare_dram_parameter("dbg", [128, 12, TB], BF16, isOutput=True)
    dbg2 = nc.declare_dram_parameter("dbg2", [128, 4, 1536], BF16, isOutput=True)
    dbg3 = nc.declare_dram_parameter("dbg3", [TB, 512], F32, isOutput=True)
    lg3 = logits.rearrange("(b t) n -> t b n", t=T)   # rows bt = 32*b + t

    with ChunkedTC(nc) as tc:
        with (
            tc.tile_pool(name="wts", bufs=1) as wts,
            tc.tile_pool(name="acts", bufs=1) as acts,
            tc.tile_pool(name="stage", bufs=2) as stage,
            tc.tile_pool(name="small", bufs=2) as small,
            tc.tile_pool(name="ps", bufs=2, space="PSUM") as ps,
            tc.tile_pool(name="ps2", bufs=1, space="PSUM") as ps2,
        ):
            idt = wts.tile([128, 128], F32)
            nc.sync.dma_start(idt[:], ident[:])
            ones1 = wts.tile([1, 128], BF16)
            nc.vector.memset(ones1[:], 1.0)
            w_ih = wts.tile([128, 2, 4, 1024], BF16, tag="bigw")
            nc.sync.dma_start(w_ih[:], wihT[:])
            w_hh = wts.tile([128, 2, 2, 1024], BF16, tag="recw")
            nc.sync.dma_start(w_hh[:], whhT[:])
            b_l = wts.tile([1, 2, 1024], BF16)
            nc.sync.dma_start(b_l[:], blstm[:])

            # ---- attention normalize + output + transpose ----
            attT = acts.tile([128, 4, 2, 128], BF16)  # [m][hw, (hwchunk?, cols (s%4,t))] -> [m][128hw, k, (s,t)]
            for m in range(4):
                a = stage.tile([128, HW], F32, tag="attld")
                nc.sync.dma_start(a[:], attm[m])
                ssum = small.tile([128, 1], F32, tag="ssum")
                nc.vector.tensor_reduce(ssum[:], a[:], mybir.AxisListType.X, mybir.AluOpType.add)
                rs = small.tile([128, 1], F32, tag="rs")
                nc.vector.reciprocal(rs[:], ssum[:])
                an = stage.tile([128, HW], F32, tag="attn")
                nc.vector.tensor_scalar_mul(an[:], a[:], rs[:])
                nc.sync.dma_start(attout[128 * m:128 * (m + 1), :], an[:])
                for k in range(2):
                    pt = ps.tile([128, 128], F32, tag="pbig")
                    nc.tensor.transpose(pt[:], an[:, 128 * k:128 * (k + 1)], idt[:])
                    nc.scalar.copy(attT[:, m, k, :], pt[:])

            # ---- pooling -> cseqT [128, (4c, tb)] bf16 ----
            cseqT = acts.tile([128, 4, TB], BF16)
            for s in range(S):
                m, sp = s // 4, s % 4
                ft = stage.tile([128, 2, C], BF16, tag="ft")
                nc.sync.dma_start(ft[:], featT[s])
                pp = ps.tile([128, 4, 32], F32, tag="pbig")
                for j in range(4):
                    for k in range(2):
                        nc.tensor.matmul(pp[:, j, :], ft[:, k, 128 * j:128 * (j + 1)],
                                         attT[:, m, k, 32 * sp:32 * (sp + 1)],
                                         start=(k == 0), stop=(k == 1))
                # cols 16*t + s, t=0..31  -> strided AP
                nc.vector.tensor_copy(cseqT[:, :, s::S].rearrange("p c t -> p (c t)"), pp[:].rearrange("p c t -> p (c t)"))

            # ---- LSTM input projection: gx[dir][mt] [128, 1024] f32 ----
            gx = acts.tile([128, 2, 4, 1024], BF16)
            for d in range(2):
                for mt in range(4):
                    for n in range(2):
                        pg = ps.tile([128, 512], F32, tag="pbig")
                        for j in range(4):
                            nc.tensor.matmul(pg[:], cseqT[:, j, 128 * mt:128 * (mt + 1)],
                                             w_ih[:, d, j, 512 * n:512 * (n + 1)],
                                             start=(j == 0), stop=False)
                        nc.tensor.matmul(pg[:], ones1[:], b_l[:, d, 512 * n:512 * (n + 1)],
                                         start=False, stop=True)
                        nc.scalar.copy(gx[:, d, mt, 512 * n:512 * (n + 1)], pg[:])

            # ---- LSTM recurrence ----
            xg = acts.tile([128, 8, TB], BF16)        # GRU input.T rows: hf(2) hb(2) pe(4)
            nc.sync.dma_start(xg[:, 4:8, :], peT[:])
            hT = acts.tile([128, 2, 2, 16], BF16)     # transposed hidden per dir
            nc.vector.memset(hT[:], 0.0)
            cst = acts.tile([16, 2, HH], F32)
            nc.vector.memset(cst[:], 0.0)
            for t in range(T):
                for d in range(2):
                    st = t if d == 0 else T - 1 - t
                    mt, tp = st // 8, st % 8
                    gxs = stage.tile([16, 1024], BF16, tag="gxs")
                    nc.sync.dma_start(gxs[:], gx[16 * tp:16 * (tp + 1), d, mt, :])
                    pg0 = ps2.tile([16, 512], F32, tag="pa")
                    pg1 = ps2.tile([16, 512], F32, tag="pb")
                    for kc in range(2):
                        nc.tensor.matmul(pg0[:], hT[:, d, kc, :], w_hh[:, d, kc, 0:512],
                                         start=(kc == 0), stop=(kc == 1))
                        nc.tensor.matmul(pg1[:], hT[:, d, kc, :], w_hh[:, d, kc, 512:1024],
                                         start=(kc == 0), stop=(kc == 1))
                    sg = small.tile([16, 1024], F32, tag="sg")
                    nc.vector.tensor_add(sg[:, 0:512], pg0[:], gxs[:, 0:512])
                    nc.vector.tensor_add(sg[:, 512:1024], pg1[:], gxs[:, 512:1024])
                    a_if = small.tile([16, 512], F32, tag="aif")
                    nc.scalar.activation(a_if[:], sg[:, 0:512], AF.Sigmoid)
                    tg = small.tile([16, 256], F32, tag="tg")
                    nc.scalar.activation(tg[:], sg[:, 512:768], AF.Tanh)
                    so = small.tile([16, 256], F32, tag="so")
                    nc.scalar.activation(so[:], sg[:, 768:1024], AF.Sigmoid)
                    c1 = small.tile([16, 256], F32, tag="c1")
                    nc.vector.tensor_mul(c1[:], a_if[:, 256:512], cst[:, d, :])
                    c2 = small.tile([16, 256], F32, tag="c2")
                    nc.vector.tensor_mul(c2[:], a_if[:, 0:256], tg[:])
                    nc.vector.tensor_add(cst[:, d, :], c1[:], c2[:])
                    tc_ = small.tile([16, 256], F32, tag="tc")
                    nc.scalar.activation(tc_[:], cst[:, d, :], AF.Tanh)
                    hsb = small.tile([16, 256], F32, tag="hsb")
                    nc.vector.tensor_mul(hsb[:], so[:], tc_[:])
                    pt = ps2.tile([128, 2, 16], F32, tag="pd")
                    for kc in range(2):
                        nc.tensor.transpose(pt[:, kc, :], hsb[:, 128 * kc:128 * (kc + 1)], idt[0:16, 0:16])
                    nc.scalar.copy(hT[:, d].rearrange("p k c -> p (k c)"), pt[:].rearrange("p k c -> p (k c)"))
                    for kc in range(2):
                        nc.scalar.copy(xg[:, 2 * d + kc, 16 * st:16 * (st + 1)], pt[:, kc, :])

            # ---- GRU input projection gi[mt] [128, 1536] f32 ----
            w_gih = wts.tile([128, 8, 1536], BF16, tag="bigw")
            nc.sync.dma_start(w_gih[:], wgihT[:])
            b_gi = wts.tile([1, 1536], BF16)
            nc.sync.dma_start(b_gi[:], bgih[:])
            w_ghh = wts.tile([128, 4, 1536], BF16, tag="recw")
            nc.sync.dma_start(w_ghh[:], wghhT[:])
            b_gh = wts.tile([1, 1536], BF16)
            nc.sync.dma_start(b_gh[:], bghh[:])
            gi = acts.tile([128, 4, 1536], BF16)
            for mt in range(4):
                for n in range(3):
                    pg = ps.tile([128, 512], F32, tag="pbig")
                    for j in range(8):
                        nc.tensor.matmul(pg[:], xg[:, j, 128 * mt:128 * (mt + 1)],
                                         w_gih[:, j, 512 * n:512 * (n + 1)],
                                         start=(j == 0), stop=False)
                    nc.tensor.matmul(pg[:], ones1[:], b_gi[:, 512 * n:512 * (n + 1)],
                                     start=False, stop=True)
                    nc.scalar.copy(gi[:, mt, 512 * n:512 * (n + 1)], pg[:])

            # ---- GRU recurrence ----
            hTg = acts.tile([128, 4, 16], BF16)
            nc.vector.memset(hTg[:], 0.0)
            hg = acts.tile([16, C], F32)
            nc.vector.memset(hg[:], 0.0)
            gres = acts.tile([128, 4, TB], BF16)
            ones16 = wts.tile([1, 16], BF16)
            nc.vector.memset(ones16[:], 1.0)
            for t in range(T):
                mt, tp = t // 8, t % 8
                gis = stage.tile([16, 1536], BF16, tag="gis")
                nc.sync.dma_start(gis[:], gi[16 * tp:16 * (tp + 1), mt, :])
                pgs = []
                for n in range(3):
                    pg = ps2.tile([16, 512], F32, tag=["pa","pb","pc"][n])
                    for kc in range(4):
                        nc.tensor.matmul(pg[:], hTg[:, kc, :], w_ghh[:, kc, 512 * n:512 * (n + 1)],
                                         start=(kc == 0), stop=False)
                    nc.tensor.matmul(pg[:], ones16[:], b_gh[:, 512 * n:512 * (n + 1)],
                                     start=False, stop=True)
                    pgs.append(pg)
                srz = small.tile([16, 1024], F32, tag="srz")
                nc.vector.tensor_add(srz[:, 0:512], pgs[0][:], gis[:, 0:512])
                nc.vector.tensor_add(srz[:, 512:1024], pgs[1][:], gis[:, 512:1024])
                rz = small.tile([16, 1024], F32, tag="rz")
                nc.scalar.activation(rz[:], srz[:], AF.Sigmoid)
                t1 = small.tile([16, 512], F32, tag="t1")
                nc.vector.tensor_mul(t1[:], rz[:, 0:512], pgs[2][:])
                t2 = small.tile([16, 512], F32, tag="t2")
                nc.vector.tensor_add(t2[:], t1[:], gis[:, 1024:1536])
                n_ = small.tile([16, 512], F32, tag="n_")
                nc.scalar.activation(n_[:], t2[:], AF.Tanh)
                dd = small.tile([16, 512], F32, tag="dd")
                nc.vector.tensor_sub(dd[:], hg[:], n_[:])
                ee = small.tile([16, 512], F32, tag="ee")
                nc.vector.tensor_mul(ee[:], rz[:, 512:1024], dd[:])
                nc.vector.tensor_add(hg[:], n_[:], ee[:])
                pt = ps2.tile([128, 4, 16], F32, tag="pd")
                for kc in range(4):
                    nc.tensor.transpose(pt[:, kc, :], hg[:, 128 * kc:128 * (kc + 1)], idt[0:16, 0:16])
                nc.scalar.copy(hTg[:].rearrange("p k c -> p (k c)"), pt[:].rearrange("p k c -> p (k c)"))
                for kc in range(4):
                    nc.scalar.copy(gres[:, kc, 16 * t:16 * (t + 1)], pt[:, kc, :])

            nc.sync.dma_start(dbg[:, 0:4, :], gres[:])
            nc.sync.dma_start(dbg2[:], gi[:])
            # ---- generator ----
            gb = wts.tile([1, NCP], BF16)
            nc.sync.dma_start(gb[:], genb[:])
            for n in range(13):
                gw = stage.tile([128, 4, 512], BF16, tag="gw")
                nc.sync.dma_start(gw[:], genwT[n])
                ncols = 512 if n < 12 else NCL - 12 * 512
                for m in range(4):
                    pg = ps.tile([128, 512], F32, tag="pbig")
                    for j in range(4):
                        nc.tensor.matmul(pg[:], gres[:, j, 128 * m:128 * (m + 1)], gw[:, j, :],
                                         start=(j == 0), stop=False)
                    nc.tensor.matmul(pg[:], ones1[:], gb[:, 512 * n:512 * (n + 1)],
                                     start=False, stop=True)
                    ot = stage.tile([128, 512], F32, tag="genout")
                    nc.vector.tensor_copy(ot[:], pg[:])
                    # rows of this m-tile: partition p = 16*(t%8) + b, t in [8m, 8m+8)
                    nc.sync.dma_start(logits[128 * m:128 * (m + 1), 512 * n:512 * n + ncols],
                                      ot[:, 0:ncols])
    return nc


_NC = None

def kernel(**inputs):
    global _NC
    feature = np.asarray(inputs["feature"], dtype=np.float32)
    attention_map = np.asarray(inputs["attention_map"], dtype=np.float32)
    text = np.asarray(inputs["text"])
    char_embeddings = np.asarray(inputs["char_embeddings"], dtype=np.float32)
    prev_idx = np.concatenate([np.zeros((B, 1), text.dtype), text[:, :-1]], axis=1)
    pe_all = char_embeddings[prev_idx]                      # [B, T, C]

    bf = lambda x: np.ascontiguousarray(x).astype(NPBF)
    wihT = np.stack([inputs["lstm_w_ih_f"].T.reshape(4, 128, 1024).transpose(1, 0, 2),
                     inputs["lstm_w_ih_b"].T.reshape(4, 128, 1024).transpose(1, 0, 2)], axis=1)
    whhT = np.stack([inputs["lstm_w_hh_f"].T.reshape(2, 128, 1024).transpose(1, 0, 2),
                     inputs["lstm_w_hh_b"].T.reshape(2, 128, 1024).transpose(1, 0, 2)], axis=1)
    blstm = np.stack([(inputs["lstm_b_ih_f"] + inputs["lstm_b_hh_f"]).reshape(1, 1024),
                      (inputs["lstm_b_ih_b"] + inputs["lstm_b_hh_b"]).reshape(1, 1024)], axis=1)
    wgihT = np.ascontiguousarray(inputs["gru_w_ih"]).T.reshape(8, 128, 1536).transpose(1, 0, 2)
    wghhT = np.ascontiguousarray(inputs["gru_w_hh"]).T.reshape(4, 128, 1536).transpose(1, 0, 2)
    genw = np.zeros((C, NCP), np.float32)
    genw[:, :NCL] = np.asarray(inputs["gen_w"]).T
    genwT = genw.reshape(4, 128, 13, 512).transpose(2, 1, 0, 3)
    genb = np.zeros((1, NCP), np.float32)
    genb[0, :NCL] = inputs["gen_b"]
    common = dict(
        wihT=bf(wihT), whhT=bf(whhT), blstm=bf(blstm), bgih=bf(inputs["gru_b_ih"].reshape(1, 1536)),
        bghh=bf(inputs["gru_b_hh"].reshape(1, 1536)), wgihT=bf(wgihT), wghhT=bf(wghhT),
        genwT=bf(genwT), genb=bf(genb), ident=np.eye(128, dtype=np.float32),
    )
    in_maps = []
    for i in range(NCORE):
        sl = slice(S * i, S * (i + 1))
        featT = feature[sl].reshape(S, C, HW).transpose(0, 2, 1).reshape(S, 2, 128, C).transpose(0, 2, 1, 3)
        attm = attention_map[sl].reshape(4, 128, HW)
        peT = pe_all[sl].transpose(2, 1, 0).reshape(C, TB).reshape(4, 128, TB).transpose(1, 0, 2)
        m = dict(common)
        m.update(featT=bf(featT), attm=np.ascontiguousarray(attm), peT=bf(peT))
        in_maps.append(m)

    if _NC is None:
        _NC = build()
    res = run_bass_kernel_spmd(_NC, in_maps, list(range(NCORE))).results
    global DBG, DBG2
    DBG = [r.get("dbg") for r in res]
    DBG2 = [r.get("dbg2") for r in res]
    global DBG3
    DBG3 = [r.get("dbg3") for r in res]
    out_res = np.concatenate(
        [np.asarray(r["logits"]).reshape(T, S, NCL).transpose(1, 0, 2).reshape(TB, NCL) for r in res], axis=0)
    out_att = np.concatenate([r["attout"] for r in res], axis=0).reshape(B * T, H, W)
    return out_res, out_att
